# revision 33
# baseline (speedup 1.0000x reference)
"""Trainium2 Bass kernel for BertSelfAttention with C_prior multiply.

Reference (per batch b):
  q/k/v = x @ W{q,k,v}.T + b{q,k,v}            -> [S, D], split into H=16 heads of W=64
  scores = q k^T / sqrt(W); mask; softmax over k
  attn = softmax(scores) * C_prior[b]
  out = attn @ v                               -> [B, S, D]

Shapes: B=2, S=2048, D=1024, H=16, W=64.

Sharding: 8 cores; core c owns batch b=c//4 and 4 consecutive heads
(hg=c%4 -> heads 4*hg..4*hg+3). The whole per-(b,h) score block stays local.

Device strategy (per core):
  - Host pre-transposes inputs (xT, W^T column blocks, C^T) so the device
    never transposes anything big.
  - scoresT = K Q^T computed in [k, q] layout per 128-k strip, two heads per
    strip into two [128, 1024] PSUM tiles; exp runs as one [128, 1024]
    activation per head, with the attention mask folded into the per-partition
    bias column (0 / -1e4), so e comes out pre-masked.
  - softmax denominator: DVE adds strip pairs of e (bf16), then four small
    ones-matmuls per pair accumulate the column sums into a [128, 1024] PSUM
    tile (replicated across each head's 64 w-partitions). This halves the
    denominator's Tensor-engine rows vs a per-strip mask-matmul. The
    denominator matmuls trail the pair-adds by 2 strips so the in-order PE
    queue never head-of-line blocks on the DVE.
  - attn*C: ct is DMA'd duplicated as [128, 2048] = [ct|ct] (gpsimd DGE
    queue, 3 strips ahead) so one DVE multiply covers both heads.
  - A@V accumulates po over strips (64x64 column-pair matmuls that co-run on
    disjoint PE quadrants); out = po * reciprocal(denom); the host transposes
    the small per-head result on gather.
  - QKV projections are spread across phases with a deadline schedule to even
    out the Tensor-engine load; input DMAs are split across the SP/ACT/gpsimd
    DGE queues because DMA configs serialize (~650 ns each) per queue.
  - The tail after the last scores strip collapses the passA->passB stagger.
"""

import os

import numpy as np
import ml_dtypes

B, S, D, H, W = 2, 2048, 1024, 16, 64
NCORES = 8
HEADS_PER_CORE = 4
P = 128
QH = S // 2  # q processed in two halves of 1024 (phases)
NK = S // P  # 16 k-strips
BOFF = 4  # pass B trails pass A by 4 strips

_prog_cache = {}


def _build_program():
    import concourse.mybir as mybir
    import concourse.tile as tile
    from concourse import bacc

    dt = mybir.dt
    f32, bf16 = dt.float32, dt.bfloat16
    Alu = mybir.AluOpType
    Act = mybir.ActivationFunctionType

    nc = bacc.Bacc("TRN2", target_bir_lowering=False)

    xT_d = nc.declare_dram_parameter("xT", [D, S], bf16, isOutput=False)
    wqk_d = nc.declare_dram_parameter("wqk", [D, 512], bf16, isOutput=False)
    wv_d = nc.declare_dram_parameter("wv", [D, 256], bf16, isOutput=False)
    bqk_d = nc.declare_dram_parameter("bqk", [P, 4], f32, isOutput=False)
    bvr_d = nc.declare_dram_parameter("bvr", [P, 256], f32, isOutput=False)
    ct_d = nc.declare_dram_parameter("ct", [S, S], bf16, isOutput=False)
    ones_d = nc.declare_dram_parameter("onesw", [P, 64], bf16, isOutput=False)
    mkb_d = nc.declare_dram_parameter("mkb", [P, NK], f32, isOutput=False)
    out_d = nc.declare_dram_parameter("out", [256, S], f32, isOutput=True)

    # phase p -> (qh, pr); pr-major so pair-1 projections are needed late.
    PHASES = [(0, 0), (1, 0), (0, 1), (1, 1)]

    with tile.TileContext(nc) as tc:
        with tc.tile_pool(name="persist", bufs=1) as persist:
            qk_all = persist.tile([P, 4, S], bf16)
            v_sb = persist.tile([P, NK, 256], bf16)
            bqk_sb = persist.tile([P, 4], f32)
            bvr_sb = persist.tile([P, 256], f32)
            ones_sb = persist.tile([P, 64], bf16)
            mkb_sb = persist.tile([P, NK], f32)
            xT_sb = persist.tile([P, 8, S], bf16)
            wqk_sb = persist.tile([P, 8, 512], bf16)
            wv_sb = persist.tile([P, 8, 256], bf16)
            # DMA configs serialize per engine queue (~650 ns each); split the
            # input loads across the SP and (otherwise idle) gpsimd queues so
            # the first projection can start ASAP.
            xT_r = xT_d.rearrange("(o p) q -> p o q", p=P)
            wqk_r = wqk_d.rearrange("(o p) m -> p o m", p=P)
            wv_r = wv_d.rearrange("(o p) m -> p o m", p=P)
            for dc in range(8):
                weng = nc.sync if dc < 4 else nc.scalar
                weng.dma_start(out=wqk_sb[:, dc, :], in_=wqk_r[:, dc, :])
                nc.gpsimd.dma_start(out=xT_sb[:, dc, :], in_=xT_r[:, dc, :])
            nc.sync.dma_start(out=bqk_sb[:], in_=bqk_d[:])
            nc.sync.dma_start(out=bvr_sb[:], in_=bvr_d[:])
            nc.sync.dma_start(out=ones_sb[:], in_=ones_d[:])
            nc.sync.dma_start(out=mkb_sb[:], in_=mkb_d[:])
            nc.sync.dma_start(out=wv_sb[:], in_=wv_r[:])

            with tc.tile_pool(name="ep", bufs=4) as ep, tc.tile_pool(
                name="ap", bufs=6
            ) as app, tc.tile_pool(name="ctp", bufs=6) as ctp, tc.tile_pool(
                name="pep", bufs=3
            ) as pep, tc.tile_pool(
                name="small", bufs=2
            ) as smallp, tc.tile_pool(
                name="psp", bufs=2, space="PSUM"
            ) as psp, tc.tile_pool(
                name="pop", bufs=1, space="PSUM"
            ) as pop, tc.tile_pool(
                name="prsp", bufs=1, space="PSUM"
            ) as prsp:

                def proj_qk(*calls):
                    # up to two (col, qb) projections share one PSUM tile to
                    # halve allocation churn on the 2-slot ring
                    ps = psp.tile([P, 1024], f32, tag="ps")
                    for i, (col, qb) in enumerate(calls):
                        for dc in range(8):
                            nc.tensor.matmul(
                                ps[:, i * 512 : (i + 1) * 512],
                                lhsT=wqk_sb[:, dc, col * P : (col + 1) * P],
                                rhs=xT_sb[:, dc, qb * 512 : (qb + 1) * 512],
                                start=(dc == 0),
                                stop=(dc == 7),
                            )
                    for i, (col, qb) in enumerate(calls):
                        # copy+bias on DVE: ACT paces the exp chain
                        nc.vector.tensor_scalar_add(
                            out=qk_all[:, col, qb * 512 : (qb + 1) * 512],
                            in0=ps[:, i * 512 : (i + 1) * 512],
                            scalar1=bqk_sb[:, col : col + 1],
                        )

                def proj_v(*kts):
                    # two consecutive k-strips share one PSUM tile
                    ps = psp.tile([P, 1024], f32, tag="ps")
                    for i, kt in enumerate(kts):
                        for dc in range(8):
                            nc.tensor.matmul(
                                ps[:, i * 256 : (i + 1) * 256],
                                lhsT=xT_sb[:, dc, kt * P : (kt + 1) * P],
                                rhs=wv_sb[:, dc, :],
                                start=(dc == 0),
                                stop=(dc == 7),
                            )
                    for i, kt in enumerate(kts):
                        nc.vector.tensor_tensor(
                            v_sb[:, kt, :],
                            ps[:, i * 256 : (i + 1) * 256],
                            bvr_sb[:],
                            Alu.add,
                        )

                # deadline-spread projection schedule (see design note):
                # pre-loop: col0 qb0, col0 qb1, col1 qb0
                # iter -> list of (col, qb) groups; pairs share a PSUM tile
                proj_sched = {
                    0: [(1, 1)],
                    1: [(1, 2)],
                    2: [(1, 3)],
                    3: [(0, 2)],
                    5: [(0, 3)],
                    17: [(3, 0)],
                    19: [(3, 1)],
                    21: [(3, 2)],
                    23: [(3, 3)],
                    25: [(2, 0)],
                    27: [(2, 1)],
                    33: [(2, 2)],
                    35: [(2, 3)],
                }
                projv_by_iter = {ks: (ks,) for ks in range(NK)}
                # warm the ACT engine: trigger the 1.3us EXP table load now,
                # under the input DMAs, not at the first real exp
                warm = smallp.tile([P, 8], f32, tag="warm")
                nc.vector.memset(warm[:], 0.0)
                nc.scalar.activation(warm[:], warm[:], Act.Exp, scale=1.0)

                proj_qk((0, 0))
                proj_qk((1, 0))
                proj_qk((0, 1))

                def passA(s):
                    ph, ks = divmod(s, NK)
                    qh, pr = PHASES[ph]
                    qcol, kcol = (0, 1) if pr == 0 else (2, 3)
                    e_t = ep.tile([P, 2048], bf16, tag="e")
                    # emit the two heads' matmuls interleaved: adjacent
                    # row-group-disjoint matmuls stream on separate XBUSes
                    # and can co-run on the PE array
                    pss = [
                        psp.tile([P, 1024], f32, tag="ps", name=f"sc{hi}")
                        for hi in range(2)
                    ]
                    for q2 in range(2):
                        qs = slice(qh * QH + q2 * 512, qh * QH + (q2 + 1) * 512)
                        for hi, (plo, phi2) in enumerate(((0, 64), (64, 128))):
                            nc.tensor.matmul(
                                pss[hi][:, q2 * 512 : (q2 + 1) * 512],
                                lhsT=qk_all[plo:phi2, kcol, ks * P : (ks + 1) * P],
                                rhs=qk_all[plo:phi2, qcol, qs],
                                tile_position=(plo, 0),
                                start=True,
                                stop=True,
                            )
                    for hi in range(2):
                        nc.scalar.activation(
                            e_t[:, hi * 1024 : (hi + 1) * 1024],
                            pss[hi][:],
                            Act.Exp,
                            bias=mkb_sb[:, ks : ks + 1],
                            scale=0.125,
                        )
                    return e_t

                e_tiles = {}
                a_tiles = {}
                ct_tiles = {}
                po_t = {}
                prs_t = {}
                rcs_t = {}

                pe_tiles = {}

                def issue_ct(s):
                    ph, ks = divmod(s, NK)
                    qh, _ = PHASES[ph]
                    ct = ctp.tile([P, 2048], bf16, tag="ct")
                    src = ct_d[ks * P : (ks + 1) * P, qh * QH : (qh + 1) * QH]
                    nc.gpsimd.dma_start(out=ct[:, 0:1024], in_=src)
                    nc.gpsimd.dma_start(out=ct[:, 1024:2048], in_=src)
                    ct_tiles[s] = ct

                issue_ct(0)
                issue_ct(1)
                issue_ct(2)

                for g in range(4 * NK):
                    if g < 4 * NK:
                        s = g
                        ph, ks = divmod(s, NK)
                        if s + 3 < 4 * NK:
                            issue_ct(s + 3)
                        e_tiles[s] = passA(s)

                    # C-multiply + pair-accumulate, one strip behind
                    if 1 <= g <= 4 * NK:
                        s1 = g - 1
                        ph1, ks1 = divmod(s1, NK)
                        a_t = app.tile([P, 2048], bf16, tag="a")
                        nc.vector.tensor_tensor(
                            a_t[:], e_tiles[s1][:], ct_tiles[s1][:], Alu.mult
                        )
                        a_tiles[s1] = a_t
                        del ct_tiles[s1]
                        if ks1 % 2 == 1:
                            pe_t = pep.tile([P, 2048], bf16, tag="pe")
                            nc.vector.tensor_tensor(
                                pe_t[:],
                                e_tiles[s1 - 1][:],
                                e_tiles[s1][:],
                                Alu.add,
                            )
                            pe_tiles[s1] = pe_t
                            del e_tiles[s1 - 1]
                            del e_tiles[s1]

                    # denominator matmuls, two strips behind the pair-add so
                    # the PE never head-of-line blocks on the DVE add
                    if 3 <= g <= 4 * NK + 2:
                        s3 = g - 3
                        ph3, ks3 = divmod(s3, NK)
                        if ks3 % 2 == 1:
                            m = ks3 // 2
                            pe_t = pe_tiles.pop(s3)
                            if m == 0:
                                prs_t[ph3] = prsp.tile(
                                    [P, 1024], f32, tag="prs", name="prs"
                                )
                            prs = prs_t[ph3]
                            for q2 in range(2):
                                for hi in range(2):
                                    nc.tensor.matmul(
                                        prs[
                                            hi * 64 : (hi + 1) * 64,
                                            q2 * 512 : (q2 + 1) * 512,
                                        ],
                                        lhsT=ones_sb[:],
                                        rhs=pe_t[
                                            :,
                                            hi * 1024
                                            + q2 * 512 : hi * 1024
                                            + (q2 + 1) * 512,
                                        ],
                                        tile_position=(0, hi * 64),
                                        start=(m == 0),
                                        stop=(m == 7),
                                    )
                            if m == 7:
                                # denominators complete: reciprocal now so the
                                # prs bank frees before the next phase needs it
                                rcs = smallp.tile([P, 1024], f32, tag="rcs")
                                scr = smallp.tile([P, 1024], f32, tag="scr")
                                nc.vector.reciprocal_approx_accurate(
                                    rcs[:], prs[:], scr[:]
                                )
                                rcs_t[ph3] = rcs
                                del prs_t[ph3]

                    # A@V side, BOFF strips behind
                    if g >= BOFF:
                        s4 = g - BOFF
                        ph4, ks4 = divmod(s4, NK)
                        qh4, pr4 = PHASES[ph4]
                        h0, h1 = 2 * pr4, 2 * pr4 + 1
                        if ks4 == 0:
                            po_t[ph4] = pop.tile([P, 1024], f32, tag="po", name="po")
                        po = po_t[ph4]
                        a_t = a_tiles[s4]
                        for q2 in range(2):
                            for hi, h in enumerate((h0, h1)):
                                nc.tensor.matmul(
                                    po[
                                        hi * 64 : (hi + 1) * 64,
                                        q2 * 512 : (q2 + 1) * 512,
                                    ],
                                    lhsT=v_sb[:, ks4, h * 64 : (h + 1) * 64],
                                    rhs=a_t[
                                        :,
                                        hi * 1024
                                        + q2 * 512 : hi * 1024
                                        + (q2 + 1) * 512,
                                    ],
                                    tile_position=(0, hi * 64),
                                    start=(ks4 == 0),
                                    stop=(ks4 == NK - 1),
                                )
                        del a_tiles[s4]
                        if ks4 == NK - 1:
                            ob = smallp.tile([P, 1024], f32, tag="ob")
                            nc.vector.tensor_tensor(
                                ob[:], po[:], rcs_t[ph4][:], Alu.mult
                            )
                            nc.sync.dma_start(
                                out=out_d[
                                    pr4 * P : (pr4 + 1) * P,
                                    qh4 * QH : (qh4 + 1) * QH,
                                ],
                                in_=ob[:],
                            )
                            del rcs_t[ph4]
                            del po_t[ph4]

                    # projections last: slack-filler for the PE queue
                    if g < 4 * NK:
                        if g in projv_by_iter:
                            proj_v(*projv_by_iter[g])
                        if g in proj_sched:
                            proj_qk(*proj_sched[g])

                # compact tail: no more scores work, so collapse the stagger
                def cmult(s1):
                    a_t = app.tile([P, 2048], bf16, tag="a")
                    nc.vector.tensor_tensor(
                        a_t[:], e_tiles[s1][:], ct_tiles[s1][:], Alu.mult
                    )
                    a_tiles[s1] = a_t
                    del ct_tiles[s1]

                def av(s4):
                    ph4, ks4 = divmod(s4, NK)
                    qh4, pr4 = PHASES[ph4]
                    h0, h1 = 2 * pr4, 2 * pr4 + 1
                    po = po_t[ph4]
                    a_t = a_tiles[s4]
                    for q2 in range(2):
                        for hi, h in enumerate((h0, h1)):
                            nc.tensor.matmul(
                                po[
                                    hi * 64 : (hi + 1) * 64,
                                    q2 * 512 : (q2 + 1) * 512,
                                ],
                                lhsT=v_sb[:, ks4, h * 64 : (h + 1) * 64],
                                rhs=a_t[
                                    :,
                                    hi * 1024
                                    + q2 * 512 : hi * 1024
                                    + (q2 + 1) * 512,
                                ],
                                tile_position=(0, hi * 64),
                                start=(ks4 == 0),
                                stop=(ks4 == NK - 1),
                            )
                    del a_tiles[s4]

                def denom_pair(s3):
                    ph3, ks3 = divmod(s3, NK)
                    m = ks3 // 2
                    pe_t = pe_tiles.pop(s3)
                    prs = prs_t[ph3]
                    for q2 in range(2):
                        for hi in range(2):
                            nc.tensor.matmul(
                                prs[
                                    hi * 64 : (hi + 1) * 64,
                                    q2 * 512 : (q2 + 1) * 512,
                                ],
                                lhsT=ones_sb[:],
                                rhs=pe_t[
                                    :,
                                    hi * 1024
                                    + q2 * 512 : hi * 1024
                                    + (q2 + 1) * 512,
                                ],
                                tile_position=(0, hi * 64),
                                start=(m == 0),
                                stop=(m == 7),
                            )

                if 63 not in a_tiles:
                    cmult(63)
                # pending pair-add (strips 62,63)
                if 63 not in pe_tiles and 63 in e_tiles:
                    pe_t7 = pep.tile([P, 2048], bf16, tag="pe")
                    nc.vector.tensor_tensor(
                        pe_t7[:], e_tiles[62][:], e_tiles[63][:], Alu.add
                    )
                    pe_tiles[63] = pe_t7
                    del e_tiles[62]
                    del e_tiles[63]
                # pending denominators, then reciprocal, then pending AVs
                for s3 in sorted(pe_tiles):
                    denom_pair(s3)
                rcs = smallp.tile([P, 1024], f32, tag="rcs")
                scr = smallp.tile([P, 1024], f32, tag="scr")
                nc.vector.reciprocal_approx_accurate(rcs[:], prs_t[3][:], scr[:])
                rcs_t[3] = rcs
                del prs_t[3]
                for s4 in sorted(a_tiles):
                    ph4 = s4 // NK
                    if s4 % NK == 0:
                        po_t[ph4] = pop.tile([P, 1024], f32, tag="po", name="po")
                    av(s4)
                    if s4 % NK == NK - 1:
                        ob = smallp.tile([P, 1024], f32, tag="ob")
                        nc.vector.tensor_tensor(
                            ob[:], po_t[ph4][:], rcs_t[ph4][:], Alu.mult
                        )
                        qh4, pr4 = PHASES[ph4]
                        nc.sync.dma_start(
                            out=out_d[
                                pr4 * P : (pr4 + 1) * P,
                                qh4 * QH : (qh4 + 1) * QH,
                            ],
                            in_=ob[:],
                        )

    nc.finalize()
    return nc


def _get_program():
    if "nc" not in _prog_cache:
        _prog_cache["nc"] = _build_program()
    return _prog_cache["nc"]


def kernel(x, attention_mask, C_prior, Wq, bq, Wk, bk, Wv, bv):
    from concourse.bass_utils import run_bass_kernel_spmd

    x = np.asarray(x, dtype=np.float32)
    attention_mask = np.asarray(attention_mask)
    C_prior = np.asarray(C_prior, dtype=np.float32)
    Wq = np.asarray(Wq, dtype=np.float32)
    Wk = np.asarray(Wk, dtype=np.float32)
    Wv = np.asarray(Wv, dtype=np.float32)
    bq = np.asarray(bq, dtype=np.float32)
    bk = np.asarray(bk, dtype=np.float32)
    bv = np.asarray(bv, dtype=np.float32)
    bf = ml_dtypes.bfloat16

    WqT, WkT, WvT = Wq.T, Wk.T, Wv.T  # [in D, out D]
    maskf = attention_mask.astype(np.float32)  # [B, S]

    in_maps = []
    for c in range(NCORES):
        b, hg = c // 4, c % 4
        heads = [4 * hg + i for i in range(HEADS_PER_CORE)]
        xT = np.ascontiguousarray(x[b].T).astype(bf)  # [D, S]

        wqk = np.empty((D, 512), np.float32)
        bqk = np.zeros((P, 4), np.float32)
        for pr in range(2):
            h0, h1 = heads[2 * pr], heads[2 * pr + 1]
            wqk[:, (2 * pr) * P : (2 * pr) * P + 64] = WqT[:, h0 * 64 : h0 * 64 + 64]
            wqk[:, (2 * pr) * P + 64 : (2 * pr + 1) * P] = WqT[
                :, h1 * 64 : h1 * 64 + 64
            ]
            wqk[:, (2 * pr + 1) * P : (2 * pr + 1) * P + 64] = WkT[
                :, h0 * 64 : h0 * 64 + 64
            ]
            wqk[:, (2 * pr + 1) * P + 64 : (2 * pr + 2) * P] = WkT[
                :, h1 * 64 : h1 * 64 + 64
            ]
            bqk[0:64, 2 * pr] = bq[h0 * 64 : h0 * 64 + 64]
            bqk[64:128, 2 * pr] = bq[h1 * 64 : h1 * 64 + 64]
            bqk[0:64, 2 * pr + 1] = bk[h0 * 64 : h0 * 64 + 64]
            bqk[64:128, 2 * pr + 1] = bk[h1 * 64 : h1 * 64 + 64]

        wv = np.ascontiguousarray(WvT[:, heads[0] * 64 : (heads[-1] + 1) * 64]).astype(
            bf
        )
        bvr = np.ascontiguousarray(
            np.broadcast_to(
                bv[heads[0] * 64 : (heads[-1] + 1) * 64][None, :], (P, 256)
            )
        )
        m = maskf[b]  # [S]
        ct = (C_prior[b].T * m[:, None]).astype(bf)  # [S(k), S(q)] * mask[k]
        # exp bias columns: 0 where mask==1, -1e4 (exp underflows to 0) where 0
        mkb = np.where(
            m.reshape(NK, P).T > 0.5, np.float32(0.0), np.float32(-1e4)
        ).astype(np.float32)  # [P, NK]
        onesw = np.ones((P, 64), bf)

        in_maps.append(
            {
                "xT": xT,
                "wqk": wqk.astype(bf),
                "wv": wv,
                "bqk": bqk,
                "bvr": bvr,
                "ct": ct,
                "onesw": onesw,
                "mkb": mkb,
            }
        )

    nc = _get_program()
    trace = bool(int(os.environ.get("BASS_KERNEL_TRACE", "0")))
    res = run_bass_kernel_spmd(nc, in_maps, list(range(NCORES)), trace=trace)
    if trace:
        print(f"HW exec time: {res.exec_time_ns} ns")
        _prog_cache["last_exec_time_ns"] = res.exec_time_ns
        _prog_cache["last_trace"] = res.instructions_and_trace

    out = np.empty((B, S, D), np.float32)
    for c in range(NCORES):
        b, hg = c // 4, c % 4
        co = res.results[c]["out"]  # [256, S]
        for i in range(HEADS_PER_CORE):
            h = 4 * hg + i
            out[b, :, h * 64 : (h + 1) * 64] = co[i * 64 : (i + 1) * 64, :].T
    return out
